# revision 18
# baseline (speedup 1.0000x reference)
"""LSTM-pool kernel for Trainium2, 8-core data-parallel SPMD.

Math (per batch row b):
  x_t = [seq[b,t], seq_e[b,t], seq_t[b,t]]              (A = 384)
  z_t = x_t @ Wi + h_{t-1} @ Wh + bh                    (4F = 512, gates i,f,o,g)
  c_t = sig(f)*c_{t-1} + sig(i)*tanh(g);  h_t = sig(o)*tanh(c_t)
  out = relu([h_T, src] @ W1 + b1) @ W2 + b2

Device layout: transposed (feature on partitions, batch on the free dim).

The DMA pool serializes loads against xbar transposes (measured: zero
overlap), so transposed bytes are pure serial cost.  seq|seq_e therefore
go through the xbar as *fp8 pairs in fake-bf16 cells* (half the bytes):
cast f32->fp8e4 during the SWDGE load into an interleaved staging tile
([s, t, {seq|seq_e}, f]), transpose 16-bit cells, and the result has
feature-pairs on partitions (seq pairs on 0:64, seq_e on 64:128) --
the [K/2, 2, N] operand layout of a K=256 DoubleRow matmul, with Wi
staged pair-interleaved to match.  The interleaved rhs streams 1 fp8
col/cycle so the DR matmul costs the same cycles as the two bf16
matmuls it replaces -- the win is the halved transpose traffic.
seq_t stays bf16 (a third fp8 tensor has no stacking partner; a
half-array DoubleRow would stream at half rate).

The input projection u_{t+1} = x_{t+1} @ Wi runs one step ahead of the
recurrence at N=512 into a ping-pong PSUM tile [128, 4 quad, 512 b]
(4 banks each); the recurrent Wh matmuls (bf16) accumulate into the
same PSUM regions per half-batch, so z = u + Wh h needs no separate
add.  Gate math is bf16 end-to-end on DVE (2x mode); ACT per half-step:
sigmoid over [i|f|o] (768 free, PSUM src), tanh(g), tanh(c).  Two
half-batches are staggered so ACT/DVE of one half hide under the
matmuls of the other.
"""

import sys

sys.path.insert(0, "/opt/trn_rl_repo")

import numpy as np

import concourse.bass as bass
import concourse.mybir as mybir
import concourse.tile as tile
from concourse import bacc
from concourse.bass_utils import run_bass_kernel_spmd

dt = mybir.dt
AF = mybir.ActivationFunctionType
DR = mybir.MatmulPerfMode.DoubleRow

NCORES = 8
BFULL = 4096
B = BFULL // NCORES  # 512 batch rows per core
T = 128
F = 128
A = 384
G = 512  # 4F
TC = 16  # time steps per DMA chunk
NH = B // 2  # half-batch = 256

# z quad order along the PSUM free dim: [i | f | o | g] so one sigmoid op
# covers quads 0..2 and tanh covers quad 3.  Column offsets into Wi/Wh.
QUADS = [("i", 0, 0), ("f", 1, 128), ("o", 2, 384), ("g", 3, 256)]


def build_nc(zero_bias: bool, t_steps: int = T):
    nc = bacc.Bacc("TRN2", target_bir_lowering=False, debug=False, num_devices=NCORES)

    seq = nc.dram_tensor("seq", [B, T, F], dt.float32, kind="ExternalInput")
    seq_e = nc.dram_tensor("seq_e", [B, T, F], dt.float32, kind="ExternalInput")
    seq_t = nc.dram_tensor("seq_t", [B, T, F], dt.float32, kind="ExternalInput")
    src = nc.dram_tensor("src", [B, F], dt.float32, kind="ExternalInput")
    Wi = nc.dram_tensor("Wi", [A, G], dt.float32, kind="ExternalInput")
    Wh = nc.dram_tensor("Wh", [F, G], dt.float32, kind="ExternalInput")
    bh = nc.dram_tensor("bh", [G], dt.float32, kind="ExternalInput")
    W1 = nc.dram_tensor("W1", [2 * F, F], dt.float32, kind="ExternalInput")
    b1 = nc.dram_tensor("b1", [F], dt.float32, kind="ExternalInput")
    W2 = nc.dram_tensor("W2", [F, F], dt.float32, kind="ExternalInput")
    b2 = nc.dram_tensor("b2", [F], dt.float32, kind="ExternalInput")
    outT = nc.dram_tensor("outT", [F, B], dt.float32, kind="ExternalOutput")

    nchunk = t_steps // TC

    with tile.TileContext(nc) as tc:
        with (
            tc.tile_pool(name="const", bufs=1) as constp,
            tc.tile_pool(name="stage", bufs=2) as stagep,
            tc.tile_pool(name="xt", bufs=3) as xtp,
            tc.tile_pool(name="gates", bufs=1) as gatep,
        ):
            # ------------- weights -------------
            # Wi seq/seq_e rows as fp8, pair-interleaved [c, i, g]: the
            # DoubleRow contraction index is k(c,i); partitions 0:64 carry
            # seq rows 2c+i, 64:128 carry seq_e rows.
            wi_view = Wi[:].rearrange("(kc c i) g -> kc c i g", kc=3, i=2)
            wi_se = constp.tile([128, 2, G], dt.float8e4, name="wi_se")
            nc.gpsimd.dma_start(wi_se[0:64, :, :], wi_view[0])
            nc.gpsimd.dma_start(wi_se[64:128, :, :], wi_view[1])
            # Wi seq_t rows as bf16 (plain matmul)
            wi_t = constp.tile([128, G], dt.bfloat16, name="wi_t")
            nc.gpsimd.dma_start(
                wi_t[:], Wi[:].rearrange("(kc k) g -> kc k g", k=128)[2]
            )

            wh_bf = constp.tile([128, G], dt.bfloat16)
            nc.gpsimd.dma_start(wh_bf[:], Wh[:])
            w1_bf = constp.tile([128, 2, F], dt.bfloat16)
            nc.gpsimd.dma_start(
                w1_bf[:], W1[:].rearrange("(kc k) m -> k kc m", k=128)
            )
            w2_bf = constp.tile([128, F], dt.bfloat16)
            nc.gpsimd.dma_start(w2_bf[:], W2[:])
            b1t = constp.tile([128, 1], dt.float32)
            nc.sync.dma_start(b1t[:], b1[:].rearrange("(f one) -> f one", one=1))
            b2t = constp.tile([128, 1], dt.float32)
            nc.sync.dma_start(b2t[:], b2[:].rearrange("(f one) -> f one", one=1))

            if not zero_bias:
                # bh folded into u via a rank-1 matmul: u += ones^T @ bh_row.
                bh_row = constp.tile([1, G], dt.bfloat16)
                nc.gpsimd.dma_start(
                    bh_row[:], bh[:].rearrange("(one g) -> one g", one=1)
                )
                ones_row = constp.tile([1, B], dt.bfloat16)
                nc.gpsimd.memset(ones_row[:], 1.0)

            # src^T (bf16): cast-DMA then xbar transpose (tiny; done upfront).
            # src_bm shares the merge-phase hid slot (same 1 KiB footprint).
            src_bm = gatep.tile([128, 4, F], dt.bfloat16, tag="hid", name="src_bm")
            nc.gpsimd.dma_start(
                src_bm[:], src[:].rearrange("(s p) f -> p s f", p=128)
            )
            srcT = constp.tile([128, 4, 128], dt.bfloat16)
            nc.sync.dma_start_transpose(
                srcT[:], src_bm[:].rearrange("p s f -> p (s f)")
            )

            # ---------------- persistent state (bf16, SBUF) ----------------
            cs = []
            hs = []
            for h in range(2):
                c_h = constp.tile([128, NH], dt.bfloat16, name=f"c_{h}")
                nc.gpsimd.memset(c_h[:], 0.0)
                cs.append(c_h)
                h_h = constp.tile([128, NH], dt.bfloat16, name=f"h_{h}")
                nc.gpsimd.memset(h_h[:], 0.0)
                hs.append(h_h)

            # ---------------- main loop ----------------
            # z ping-pong: [128, 4 quads, 512 b] fp32 = 4 PSUM banks each.
            zp_ctx = tc.tile_pool(name="zp", bufs=2, space="PSUM")
            zp = zp_ctx.__enter__()

            def z_tile(t):
                return zp.tile([128, 4, B], dt.float32, tag="z", name=f"z_{t}")

            def input_proj(z, xt_se, xt_t, ts_, quads=QUADS):
                """u = x_t @ Wi (+ bh) for the full batch, N=512."""
                rhs_se = (
                    xt_se[:, :, ts_, :]
                    .bitcast(dt.float8e4)
                    .rearrange("p s (b i) -> p i s b", i=2)
                )
                rhs_t = xt_t[:, :, ts_, :]
                for qname, qi, woff in quads:
                    nc.tensor.matmul(
                        z[:, qi, :],
                        wi_se[:, :, woff : woff + 128],
                        rhs_se,
                        start=True,
                        stop=False,
                        perf_mode=DR,
                    )
                    nc.tensor.matmul(
                        z[:, qi, :],
                        wi_t[:, woff : woff + 128],
                        rhs_t,
                        start=False,
                        stop=False,
                    )
                if not zero_bias:
                    for qname, qi, woff in quads:
                        nc.tensor.matmul(
                            z[:, qi, :],
                            bh_row[:, woff : woff + 128],
                            ones_row[:],
                            start=False,
                            stop=False,
                        )

            sgs = [None, None]

            def gates_part(z, h):
                """Wh accumulate + gate activations + cell update, half h."""
                bs = slice(h * NH, (h + 1) * NH)
                for qname, qi, woff in QUADS:
                    nc.tensor.matmul(
                        z[:, qi, bs],
                        wh_bf[:, woff : woff + 128],
                        hs[h][:],
                        start=False,
                        stop=True,
                    )
                # gates: one sigmoid over [i|f|o], one tanh over g (PSUM src)
                sg = gatep.tile(
                    [128, 3, NH], dt.bfloat16, tag=f"sg{h}", name=f"sg{h}"
                )
                nc.scalar.activation(sg[:], z[:, 0:3, bs], AF.Sigmoid)
                tg = gatep.tile([128, NH], dt.bfloat16, tag=f"tg{h}", name=f"tg{h}")
                nc.scalar.activation(tg[:], z[:, 3, bs], AF.Tanh)
                sgs[h] = sg

                # cell update (DVE, bf16 2x)
                m2 = gatep.tile([128, NH], dt.bfloat16, tag=f"m2_{h}", name=f"m2{h}")
                nc.vector.tensor_mul(m2[:], sg[:, 0, :], tg[:])
                m1 = gatep.tile([128, NH], dt.bfloat16, tag=f"m1_{h}", name=f"m1{h}")
                nc.vector.tensor_mul(m1[:], sg[:, 1, :], cs[h][:])
                nc.vector.tensor_add(cs[h][:], m1[:], m2[:])

            def h_part(h):
                """tanh(c) on ACT, then the h DVE mul (reuses the tg ring).
                Issued separately so the other half's sigmoid/tanh(g) can
                run on the strict-FIFO ACT queue while tanh(c) waits for
                the DVE cell add."""
                tc2 = gatep.tile([128, NH], dt.bfloat16, tag=f"tg{h}", name=f"tc{h}")
                nc.scalar.activation(tc2[:], cs[h][:], AF.Tanh)
                nc.vector.tensor_mul(hs[h][:], sgs[h][:, 2, :], tc2[:])

            def recur_half(z, h):
                gates_part(z, h)
                h_part(h)

            cur_z = None
            for ch in range(nchunk):
                t0 = ch * TC
                # seq|seq_e: fp8 cast loads into the pair-interleaved staging
                # tile, one DMA per (tensor, s-block) so the 3-dim AP
                # balancer accepts the strided destination.
                bm_se = stagep.tile(
                    [128, 4, TC, 2, F], dt.float8e4, tag="bm_se", name=f"bm_se_{ch}"
                )
                for s in range(4):
                    nc.gpsimd.dma_start(
                        bm_se[:, s, :, 0, :],
                        seq[:].rearrange("(s p) t f -> s p t f", p=128)[
                            s, :, t0 : t0 + TC, :
                        ],
                    )
                    nc.gpsimd.dma_start(
                        bm_se[:, s, :, 1, :],
                        seq_e[:].rearrange("(s p) t f -> s p t f", p=128)[
                            s, :, t0 : t0 + TC, :
                        ],
                    )
                bm_t = stagep.tile(
                    [128, 4, TC, F], dt.bfloat16, tag="bm_t", name=f"bm_t_{ch}"
                )
                nc.gpsimd.dma_start(
                    bm_t[:],
                    seq_t[:].rearrange("(s p) t f -> p s t f", p=128)[
                        :, :, t0 : t0 + TC, :
                    ],
                )
                # transposes: fake-bf16 fp8 pairs for seq|seq_e, plain bf16
                # for seq_t.  xt_se[:, s, t, :] = feature pairs on partitions
                # (seq 0:64 | seq_e 64:128), batch on the free dim.
                xt_se = xtp.tile(
                    [128, 4, TC, 128], dt.bfloat16, tag="xt_se", name=f"xt_se_{ch}"
                )
                nc.sync.dma_start_transpose(
                    xt_se[:],
                    bm_se[:].rearrange("p s t i f -> p (s t i f)").bitcast(
                        dt.bfloat16
                    ),
                )
                xt_t = xtp.tile(
                    [128, 4, TC, 128], dt.bfloat16, tag="xt_t", name=f"xt_t_{ch}"
                )
                nc.sync.dma_start_transpose(
                    xt_t[:], bm_t[:].rearrange("p s t f -> p (s t f)")
                )

                for ts_ in range(TC):
                    t = t0 + ts_
                    if t == 0:
                        cur_z = z_tile(0)
                        input_proj(cur_z, xt_se, xt_t, 0)
                        continue
                    # next step's input projection (independent of h), then
                    # this step's recurrence per half.  Program order sets the
                    # tensor-engine FIFO: the Wh matmuls for half A go first
                    # (critical path), then the u matmuls run while ACT/DVE
                    # chew on half A, then Wh for half B.
                    z = cur_z
                    nz = z_tile(t)
                    gates_part(z, 0)
                    input_proj(nz, xt_se, xt_t, ts_, QUADS[:2])
                    gates_part(z, 1)
                    h_part(0)
                    input_proj(nz, xt_se, xt_t, ts_, QUADS[2:])
                    h_part(1)
                    cur_z = nz

            # final step's recurrence
            recur_half(cur_z, 0)
            recur_half(cur_z, 1)

            zp_ctx.__exit__(None, None, None)

            # ---------------- merge layer ----------------
            with tc.tile_pool(name="mp", bufs=1, space="PSUM") as mp:
                ps_hid = mp.tile([128, B], dt.float32)
                for h in range(2):
                    nc.tensor.matmul(
                        ps_hid[:, h * NH : (h + 1) * NH],
                        w1_bf[:, 0, :],
                        hs[h][:],
                        start=True,
                        stop=False,
                    )
                    nc.tensor.matmul(
                        ps_hid[:, h * NH : (h + 1) * NH],
                        w1_bf[:, 1, :],
                        srcT[:, 2 * h : 2 * h + 2, :],
                        start=False,
                        stop=True,
                    )
                hid_bf = gatep.tile([128, B], dt.bfloat16, tag="hid", name="hid")
                nc.scalar.activation(hid_bf[:], ps_hid[:], AF.Relu, bias=b1t[:])

                ps_out = mp.tile([128, B], dt.float32)
                nc.tensor.matmul(ps_out[:], w2_bf[:], hid_bf[:], start=True, stop=True)
                out_sb = constp.tile([128, B], dt.bfloat16)
                nc.scalar.activation(out_sb[:], ps_out[:], AF.Identity, bias=b2t[:])
                nc.gpsimd.dma_start(outT[:], out_sb[:])

    nc.compile()
    return nc


_NC_CACHE: dict = {}


def _get_nc(zero_bias: bool):
    if zero_bias not in _NC_CACHE:
        _NC_CACHE[zero_bias] = build_nc(zero_bias)
    return _NC_CACHE[zero_bias]


def make_in_maps(**inputs):
    """Slice full inputs into per-core input maps (batch data-parallel)."""
    f32 = lambda x: np.ascontiguousarray(np.asarray(x), dtype=np.float32)
    shared = {
        k: f32(inputs[k]) for k in ("Wi", "Wh", "bh", "W1", "b1", "W2", "b2")
    }
    in_maps = []
    for c in range(NCORES):
        sl = slice(c * B, (c + 1) * B)
        m = dict(shared)
        m["seq"] = f32(inputs["seq"][sl])
        m["seq_e"] = f32(inputs["seq_e"][sl])
        m["seq_t"] = f32(inputs["seq_t"][sl])
        m["src"] = f32(inputs["src"][sl])
        in_maps.append(m)
    return in_maps


def kernel(**inputs) -> np.ndarray:
    zero_bias = not np.any(np.asarray(inputs["bh"]))
    nc = _get_nc(zero_bias)
    in_maps = make_in_maps(**inputs)
    res = run_bass_kernel_spmd(nc, in_maps, core_ids=list(range(NCORES)))
    out = np.empty((BFULL, F), np.float32)
    for c in range(NCORES):
        out[c * B : (c + 1) * B] = res.results[c]["outT"].T
    return out
